# revision 12
# baseline (speedup 1.0000x reference)
"""DPA3 layer (nn_DPA3NextLayer) on 8 Trainium2 NeuronCores via Bass/Tile.

Strategy:
- Shard nodes into 8 contiguous blocks; edges go to the core owning n2e;
  angles go to the core owning eij2a. All segment reductions are core-local.
- Edges sorted by n2e, angles sorted by local slot of eij2a -> segment sums
  are computed per 128-item tile with a one-hot (is_equal) matrix + PE matmul,
  then written to per-parity DRAM partial tables with an indirect scatter
  (disjoint row ranges per parity -> race-free, no RMW).
- The dimwise softmax needs no segment max (values are small; stabilizer
  cancels up to the 1e-12 eps) and no gather-back: seg(e*msg)/(seg(e)+eps).
- Gathers (node[n2a], edge[eik2a], ...) via indirect DMA from replicated /
  allgathered bf16 tables. Two AllGathers: normalized original edges (for
  sub-block 1) and [normalized updated edges | envelope] + updated nodes
  (between sub-blocks 2 and 3).
- Residual stream fp32; matmuls and gathered operands bf16.
"""
import math
import numpy as np
import ml_dtypes

import sys
if "/opt/trn_rl_repo" not in sys.path:
    sys.path.insert(0, "/opt/trn_rl_repo")

import concourse.bass as bass
import concourse.mybir as mybir
import concourse.tile as tile
from concourse import bacc
from concourse import bass_utils
from concourse.masks import make_identity

P = 128
N_DIM, E_DIM, A_DIM, R_DIM = 128, 128, 64, 12
NCORES = 8
CH = 512  # items per chunk
A_SEL, NNEI, SEL_RF = 40, 120, 10.0
A_SCALE = float((A_SEL / SEL_RF) ** -0.5)
E_SCALE = float((NNEI / SEL_RF) ** -0.5)
BF = ml_dtypes.bfloat16
F32 = np.float32


# ----------------------------------------------------------------------------
# host planning
# ----------------------------------------------------------------------------

def _blocked_i32(x, nt):
    # [P*nt] -> [P, nt] with column t = items of tile t
    return np.ascontiguousarray(x.reshape(nt, P).T).astype(np.int32)


def _blocked_f32(x, nt):
    return np.ascontiguousarray(x.reshape(nt, P).T).astype(np.float32)


def _insert_pads_span(slots, other_cols, chunk):
    """Insert pad items (slot=-1) so every 128-item tile spans <=128 slots.
    slots: [n] nondecreasing (real items). other_cols: dict name->array[n].
    Returns padded arrays (multiple of chunk)."""
    n = slots.shape[0]
    out_slots = []
    outs = {k: [] for k in other_cols}
    i = 0
    cur = []  # current tile slots
    def flushable(s):
        return (not cur) or (s - cur[0] < P)
    while i < n:
        s = slots[i]
        if flushable(s):
            cur.append(s)
            out_slots.append(s)
            for k, v in other_cols.items():
                outs[k].append(v[i])
            i += 1
            if len(cur) == P:
                cur = []
        else:
            # pad until tile boundary
            out_slots.append(-1)
            for k, v in other_cols.items():
                outs[k].append(0)
            cur.append(cur[0])  # dummy to count fill
            if len(cur) == P:
                cur = []
    slots2 = np.array(out_slots, np.int64)
    res = {k: np.array(vs) for k, vs in outs.items()}
    # pad tail to chunk multiple
    total = int(math.ceil(len(slots2) / chunk) * chunk)
    pad = total - len(slots2)
    slots2 = np.concatenate([slots2, np.full(pad, -1, np.int64)])
    for k in res:
        res[k] = np.concatenate([res[k], np.zeros(pad, res[k].dtype)])
    return slots2, res


def build_plan(n2e, n_ext2e, n2a, eij2a, eik2a, num_nodes):
    npc = num_nodes // NCORES
    n_edge = n2e.shape[0]
    core_of_edge = (n2e // npc).astype(np.int64)
    order_e = np.argsort(n2e, kind="stable")
    counts_e = np.bincount(core_of_edge, minlength=NCORES)

    slot_of_edge = np.empty(n_edge, np.int64)
    edge_lists = []
    pos = 0
    for c in range(NCORES):
        k = int(counts_e[c])
        ids = order_e[pos:pos + k]
        edge_lists.append(ids)
        slot_of_edge[ids] = np.arange(k)
        pos += k

    core_of_angle = core_of_edge[eij2a]
    akey = slot_of_edge[eij2a]
    order_a = np.lexsort((akey, core_of_angle))
    counts_a = np.bincount(core_of_angle, minlength=NCORES)

    # Per-core angle arrays with span-limiting pads
    per_core_a = []
    pos = 0
    max_alen = 0
    for c in range(NCORES):
        k = int(counts_a[c])
        aids = order_a[pos:pos + k]
        pos += k
        slots = slot_of_edge[eij2a[aids]]
        cols = {
            "aid": aids,
            "n2a": n2a[aids],
            "eik_core": core_of_edge[eik2a[aids]],
            "eik_slot": slot_of_edge[eik2a[aids]],
            "eij_slot_g": slot_of_edge[eij2a[aids]],
        }
        s2, cols2 = _insert_pads_span(slots, cols, CH)
        cols2["valid"] = np.concatenate([
            np.zeros(0, bool),
            (s2 >= 0)])
        per_core_a.append((s2, cols2))
        max_alen = max(max_alen, len(s2))
    A_max = int(math.ceil(max_alen / CH) * CH)
    # re-pad all to A_max
    for c in range(NCORES):
        s2, cols2 = per_core_a[c]
        pad = A_max - len(s2)
        s2 = np.concatenate([s2, np.full(pad, -1, np.int64)])
        for k in cols2:
            cols2[k] = np.concatenate([cols2[k], np.zeros(pad, cols2[k].dtype)])
        per_core_a[c] = (s2, cols2)

    E_max = int(math.ceil(counts_e.max() / CH) * CH)
    return dict(npc=npc, E_max=E_max, A_max=A_max, counts_e=counts_e,
                counts_a=counts_a, edge_lists=edge_lists,
                slot_of_edge=slot_of_edge, core_of_edge=core_of_edge,
                per_core_a=per_core_a)


def _scatter_plan(slots, n_tiles, trash_row):
    """Per-tile first slot + per-item scatter target rows with parity tables.
    Returns first[P?]: [n_tiles] int, scat rows [n_tiles*P] int32 (within-table
    row), parity handled by caller (tile index % 2). Rows outside
    [first_t, first_next_same_parity) -> trash."""
    BIG = 10 ** 9
    first = np.zeros(n_tiles, np.int64)
    has_real = np.zeros(n_tiles, bool)
    for t in range(n_tiles):
        ts = slots[t * P:(t + 1) * P]
        real = ts[ts >= 0]
        has_real[t] = len(real) > 0
        first[t] = real[0] if len(real) else BIG
    scat = np.zeros(n_tiles * P, np.int64)
    for t in range(n_tiles):
        if not has_real[t]:
            scat[t * P:(t + 1) * P] = trash_row
            continue
        lim = first[t + 2] if t + 2 < n_tiles else BIG
        for p in range(P):
            r = first[t] + p
            scat[t * P + p] = r if r < lim else trash_row
    first = np.where(has_real, first, 0)
    return first, scat


# ----------------------------------------------------------------------------
# weights (folded, bf16)
# ----------------------------------------------------------------------------

def fold_weights(params):
    p = {k: (np.asarray(v, F32) if not isinstance(v, dict) else
             {kk: np.asarray(vv, F32) for kk, vv in v.items()}) for k, v in params.items()}
    ones_n = np.ones(N_DIM, F32)
    W = {}

    def gmlp(g, fold):
        w_in = g["w_in"] * fold[:, None]
        w_out = g["w_out"] * g["norm"][:, None]
        return w_in.astype(BF), w_out.astype(BF)

    f1 = np.concatenate([p["line_attn_norm_a"], ones_n,
                         p["line_attn_norm_e"], p["line_attn_norm_e"]])
    W["w1_in"], W["w1_out"] = gmlp(p["line_attn_mlp"], f1)
    W["g1"] = (p["line_attn_gate"] * f1[:, None]).astype(BF)

    f2 = np.concatenate([p["atom_attn_norm_e"], p["atom_attn_norm_n"], ones_n])
    W["w2a_in"], W["w2a_out"] = gmlp(p["atom_attn_mlp"], f2)
    W["w2e_in"], W["w2e_out"] = gmlp(p["atom_attn_edge_mlp"], f2)
    W["g2"] = (p["atom_attn_src_gate"] * p["atom_attn_norm_e"][:, None]).astype(BF)

    f3 = np.concatenate([p["line_ref_norm_a"], ones_n,
                         p["line_ref_norm_e"], p["line_ref_norm_e"]])
    W["w3_in"], W["w3_out"] = gmlp(p["line_ref_mlp"], f3)
    W["w3_env"] = p["line_ref_env"].astype(BF)
    W["w3_eproj"] = p["line_ref_edge_proj"].astype(BF)
    W["w3_aproj"] = p["line_ref_angle_proj"].astype(BF)

    f4 = np.concatenate([p["atom_ref_norm_e"], p["atom_ref_norm_n"], ones_n])
    W["w4_in"], W["w4_out"] = gmlp(p["atom_ref_mlp"], f4)
    W["w4_env"] = p["atom_ref_env"].astype(BF)
    W["w4_nproj"] = p["atom_ref_node_proj"].astype(BF)
    W["w4_eproj"] = p["atom_ref_edge_proj"].astype(BF)
    return W


# ----------------------------------------------------------------------------
# bass kernel builder
# ----------------------------------------------------------------------------

BFD = mybir.dt.bfloat16
FD = mybir.dt.float32
ID = mybir.dt.int32
AF = mybir.ActivationFunctionType
OP = mybir.AluOpType



def build_kernel(E_max, A_max, npc, debug_outputs=False):
    NT_A = A_max // P
    NT_E = E_max // P
    NODE_TBL = npc + P
    NBLK_N = NODE_TBL // P
    ETBL = E_max + P

    nc = bacc.Bacc(None)

    def inp(name, shape, dt):
        return nc.dram_tensor(name, list(shape), dt, kind="ExternalInput")

    def outp(name, shape, dt):
        return nc.dram_tensor(name, list(shape), dt, kind="ExternalOutput")

    t_angle_res = inp("angle_res", [A_max, A_DIM], FD)
    t_edge_res = inp("edge_res", [E_max, N_DIM], FD)
    t_node_res = inp("node_res", [npc, N_DIM], FD)
    t_node_tbl = inp("node_tbl", [NCORES * npc, N_DIM], BFD)
    t_ext_tbl = inp("ext_tbl", [12288, N_DIM], BFD)
    t_rbf_t = inp("rbf_t", [R_DIM, E_max], BFD)

    t_a_n2a = inp("a_n2a", [P, NT_A], ID)
    t_a_eik = inp("a_eik", [P, NT_A], ID)
    t_a_eij = inp("a_eij", [P, NT_A], ID)
    t_a_slot = inp("a_slot", [P, NT_A], FD)
    t_a_first = inp("a_first", [P, NT_A], FD)
    t_a_scat = inp("a_scat", [P, NT_A], ID)
    t_a_sw = inp("a_sw_b", [P, NT_A], FD)

    t_e_gnode = inp("e_gnode", [P, NT_E], ID)
    t_e_gext = inp("e_gext", [P, NT_E], ID)
    t_e_slot = inp("e_slot", [P, NT_E], FD)
    t_e_first = inp("e_first", [P, NT_E], FD)
    t_e_scat = inp("e_scat", [P, NT_E], ID)
    t_e_sw = inp("e_sw_b", [P, NT_E], FD)

    t_iota = inp("iota", [P, P], FD)
    t_ones = inp("ones_col", [P, 1], BFD)
    t_one1 = inp("one_one", [1, 1], FD)

    wshapes = dict(
        w1_in=(448, 512), w1_out=(256, 128), g1=(448, 128),
        w2a_in=(384, 512), w2a_out=(256, 128), w2e_in=(384, 512), w2e_out=(256, 128),
        g2=(128, 128),
        w3_in=(448, 512), w3_out=(256, 128), w3_env=(12, 128), w3_eproj=(128, 128),
        w3_aproj=(128, 64),
        w4_in=(384, 512), w4_out=(256, 128), w4_env=(12, 128), w4_nproj=(128, 128),
        w4_eproj=(128, 128),
    )
    t_w = {k: inp("W_" + k, v, BFD) for k, v in wshapes.items()}

    t_accA = inp("accA", [ETBL, 256], BFD)
    t_accB = inp("accB", [ETBL, 256], BFD)
    t_naccA = inp("naccA", [NODE_TBL + P, 256], BFD)
    t_naccB = inp("naccB", [NODE_TBL + P, 256], BFD)

    t_node_out = outp("node_out", [npc, N_DIM], FD)
    t_edge_out = outp("edge_out", [E_max, N_DIM], FD)
    t_angle_out = outp("angle_out", [A_max, A_DIM], FD)

    t_tbl1_own = nc.dram_tensor("tbl1_own", [E_max, N_DIM], BFD)
    t_tbl1_full = nc.dram_tensor("tbl1_full", [NCORES * E_max, N_DIM], BFD,
                                 addr_space="Shared")
    t_tbl2_own = nc.dram_tensor("tbl2_own", [E_max, 256], BFD)
    t_tbl2_full = nc.dram_tensor("tbl2_full", [NCORES * E_max, 256], BFD,
                                 addr_space="Shared")
    t_nupd_own = nc.dram_tensor("nupd_own", [npc, N_DIM], BFD)
    t_nupd_full = nc.dram_tensor("nupd_full", [NCORES * npc, N_DIM], BFD,
                                 addr_space="Shared")
    t_eres1 = nc.dram_tensor("eres1", [E_max, N_DIM], FD)
    t_eres2 = nc.dram_tensor("eres2", [E_max, N_DIM], FD)
    t_eres3 = nc.dram_tensor("eres3", [E_max, N_DIM], FD)
    t_nres1 = nc.dram_tensor("nres1", [npc, N_DIM], FD)
    t_nodeN = nc.dram_tensor("nodeN", [NODE_TBL, N_DIM], BFD)
    t_nodeN4 = nc.dram_tensor("nodeN4", [NODE_TBL, N_DIM], BFD)

    from contextlib import ExitStack
    _es = ExitStack()
    with tile.TileContext(nc) as tc, _es:
        sbc = _es.enter_context(tc.tile_pool(name="const", bufs=1))
        sbw = _es.enter_context(tc.tile_pool(name="wts", bufs=1))
        sb = _es.enter_context(tc.tile_pool(name="work", bufs=2))
        sbq = _es.enter_context(tc.tile_pool(name="quad", bufs=5))
        sbg = _es.enter_context(tc.tile_pool(name="gath", bufs=3))
        sbf = _es.enter_context(tc.tile_pool(name="fm", bufs=2))
        psp = _es.enter_context(tc.tile_pool(name="psp", bufs=1, space="PSUM"))

        def PS(shape, tag):
            return psp.tile(shape, FD, tag=tag, name="ps_" + tag, space="PSUM")

        iota_sb = sbc.tile([P, P], FD, name="iota_sb")
        nc.sync.dma_start(out=iota_sb[:], in_=t_iota[:, :])
        ident = sbc.tile([P, P], BFD, name="ident")
        make_identity(nc, ident[:])
        ones_sb = sbc.tile([P, 1], BFD, name="ones_sb")
        nc.sync.dma_start(out=ones_sb[:], in_=t_ones[:, :])
        one1_sb = sbc.tile([1, 1], FD, name="one1_sb")
        nc.sync.dma_start(out=one1_sb[:], in_=t_one1[:, :])

        meta = {}
        for nm, t, dt, w in [("a_n2a", t_a_n2a, ID, NT_A), ("a_eik", t_a_eik, ID, NT_A),
                             ("a_eij", t_a_eij, ID, NT_A), ("a_scat", t_a_scat, ID, NT_A),
                             ("a_slot", t_a_slot, FD, NT_A), ("a_first", t_a_first, FD, NT_A),
                             ("a_sw", t_a_sw, FD, NT_A),
                             ("e_gnode", t_e_gnode, ID, NT_E), ("e_gext", t_e_gext, ID, NT_E),
                             ("e_scat", t_e_scat, ID, NT_E), ("e_slot", t_e_slot, FD, NT_E),
                             ("e_first", t_e_first, FD, NT_E), ("e_sw", t_e_sw, FD, NT_E)]:
            tl = sbc.tile([P, w], dt, name="m_" + nm)
            nc.sync.dma_start(out=tl[:], in_=t[:, :])
            meta[nm] = tl

        ksplits = {k: ([64, 128, 128, 128] if kk == 448 else None)
                   for k, (kk, mm) in wshapes.items()}
        wsb = {}
        for k, (kk, mm) in wshapes.items():
            tls = []
            off = 0
            split = ksplits[k]
            si = 0
            while off < kk:
                h = split[si] if split else min(128, kk - off)
                si += 1
                tl = sbw.tile([h, mm], BFD, name=f"w_{k}_{off}")
                nc.sync.dma_start(out=tl[:], in_=t_w[k][off:off + h, :])
                tls.append(tl)
                off += h
            wsb[k] = tls

        # ---------------- helpers ----------------
        def rownorm(x_f32, d, name):
            sq = sb.tile([P, d], FD, tag="rn_sq", name=name + "sq")
            nc.vector.tensor_tensor(out=sq[:], in0=x_f32, in1=x_f32, op=OP.mult)
            ssum = sb.tile([P, 1], FD, tag="rn_ss", name=name + "ss")
            nc.vector.reduce_sum(ssum[:], sq[:], axis=mybir.AxisListType.X)
            ms = sb.tile([P, 1], FD, tag="rn_ms", name=name + "ms")
            nc.vector.tensor_scalar(out=ms[:], in0=ssum[:], scalar1=1.0 / d,
                                    scalar2=1e-6, op0=OP.mult, op1=OP.add)
            rms = sb.tile([P, 1], FD, tag="rn_rm", name=name + "rm")
            nc.scalar.activation(rms[:], ms[:], AF.Sqrt)
            inv = sb.tile([P, 1], FD, tag="rn_iv", name=name + "iv")
            nc.vector.reciprocal(inv[:], rms[:])
            nrm = sb.tile([P, d], BFD, tag="rn_nm", name=name + "nm")
            nc.vector.tensor_scalar(out=nrm[:], in0=x_f32, scalar1=inv[:],
                                    scalar2=None, op0=OP.mult)
            return nrm

        def transpose_to(dst_ap, src_ap, h):
            pt = psp.tile([P, P], BFD, tag="tp", name="ps_tp", space="PSUM")
            nc.tensor.transpose(pt[:h, :], src_ap, ident[:, :])
            nc.scalar.copy(out=dst_ap, in_=pt[:h, :])

        def gather(dst, tblap, idx_tile, t):
            nc.gpsimd.indirect_dma_start(
                out=dst, out_offset=None, in_=tblap,
                in_offset=bass.IndirectOffsetOnAxis(ap=idx_tile[:, t:t + 1], axis=0))

        def build_sel(slot_meta, first_meta, t):
            d = sb.tile([P, 1], FD, tag="seld", name="seld")
            nc.vector.tensor_tensor(out=d[:], in0=slot_meta[:, t:t + 1],
                                    in1=first_meta[:, t:t + 1], op=OP.subtract)
            sel = sb.tile([P, P], BFD, tag="sel", name="sel")
            nc.vector.tensor_tensor(out=sel[:], in0=d[:].to_broadcast([P, P]),
                                    in1=iota_sb[:], op=OP.is_equal)
            return sel

        def gmlp_chunk(rhs_list, w_in_tiles, w_out_tiles, scale_meta, base, tag):
            """Returns msg_im tiles (item-major [128,128] bf16) x4.
            scale_meta: None or meta tile [P, NT] whose cols base..base+3 give a
            per-item factor folded into msg."""
            hps = [PS([P, CH], f"h{m}") for m in range(4)]
            nk = len(w_in_tiles)
            for ki in range(nk):
                for m in range(4):
                    nc.tensor.matmul(hps[m][:], lhsT=w_in_tiles[ki][:, m * 128:(m + 1) * 128],
                                     rhs=rhs_list[ki], start=(ki == 0), stop=(ki == nk - 1))
            prods = []
            for j in range(2):
                sg = sb.tile([P, CH], BFD, tag=f"gm_sg{j}", name=f"{tag}sg{j}")
                nc.scalar.activation(sg[:], hps[2 + j][:], AF.Silu)
                pr = sbq.tile([P, CH], BFD, tag=f"gm_pr{j}", name=f"{tag}pr{j}")
                nc.vector.tensor_tensor(out=pr[:], in0=hps[j][:], in1=sg[:], op=OP.mult)
                prods.append(pr)
            ssps = PS([1, CH], "ss")
            for j in range(2):
                sq = sb.tile([P, CH], BFD, tag=f"gm_sq{j}", name=f"{tag}sq{j}")
                nc.vector.tensor_tensor(out=sq[:], in0=prods[j][:], in1=prods[j][:],
                                        op=OP.mult)
                nc.tensor.matmul(ssps[:], lhsT=ones_sb[:], rhs=sq[:],
                                 start=(j == 0), stop=(j == 1))
            msr = sb.tile([1, CH], FD, tag="gm_ms", name=f"{tag}ms")
            nc.vector.tensor_scalar(out=msr[:], in0=ssps[:], scalar1=1.0 / 256,
                                    scalar2=1e-6, op0=OP.mult, op1=OP.add)
            rmsr = sb.tile([1, CH], FD, tag="gm_rm", name=f"{tag}rm")
            nc.scalar.activation(rmsr[:], msr[:], AF.Sqrt)
            invr = sb.tile([1, CH], FD, tag="gm_ir", name=f"{tag}ir")
            nc.vector.reciprocal(invr[:], rmsr[:])
            msgs = []
            for i in range(4):
                ip = PS([P, 1], "misc")
                nc.tensor.matmul(ip[:], lhsT=invr[:, i * 128:(i + 1) * 128],
                                 rhs=one1_sb[:], start=True, stop=True)
                iw = sb.tile([P, 1], FD, tag="gm_iw", name=f"{tag}iw")
                if scale_meta is not None:
                    nc.vector.tensor_tensor(out=iw[:], in0=ip[:],
                                            in1=scale_meta[:, base + i:base + i + 1],
                                            op=OP.mult)
                else:
                    nc.vector.tensor_copy(out=iw[:], in_=ip[:])
                mp = PS([P, P], "misc")
                for j in range(2):
                    nc.tensor.matmul(mp[:], lhsT=prods[j][:, i * 128:(i + 1) * 128],
                                     rhs=w_out_tiles[j][:], start=(j == 0), stop=(j == 1))
                mi = sbq.tile([P, P], BFD, tag="gm_mi", name=f"{tag}mi")
                nc.vector.tensor_scalar(out=mi[:], in0=mp[:], scalar1=iw[:],
                                        scalar2=None, op0=OP.mult)
                msgs.append(mi)
            return msgs

        def scatter_tile(sc_bf, scat_meta, t, tblA, tblB):
            tbl = tblA if (t % 2 == 0) else tblB
            nc.gpsimd.indirect_dma_start(
                out=tbl[:, :], out_offset=bass.IndirectOffsetOnAxis(
                    ap=scat_meta[:, t:t + 1], axis=0),
                in_=sc_bf, in_offset=None)

        # ================= P0: table1 + AG =================
        for b in range(NT_E):
            er = sb.tile([P, N_DIM], FD, tag="p0er", name="p0er")
            nc.sync.dma_start(out=er[:], in_=t_edge_res[b * P:(b + 1) * P, :])
            nm = rownorm(er[:], N_DIM, "p0")
            nc.sync.dma_start(out=t_tbl1_own[b * P:(b + 1) * P, :], in_=nm[:])
        nc.gpsimd.collective_compute(
            "AllGather", OP.bypass,
            ins=[t_tbl1_own[:, :].opt()], outs=[t_tbl1_full[:, :].opt()],
            replica_groups=[list(range(NCORES))])

        # ================= angle passes =================
        def angle_pass(pass3):
            w_in = wsb["w3_in"] if pass3 else wsb["w1_in"]
            w_out = [wsb["w3_out"][0][:], wsb["w3_out"][1][:]] if pass3 else \
                    [wsb["w1_out"][0][:], wsb["w1_out"][1][:]]
            tag = "a3" if pass3 else "a1"
            for ci in range(A_max // CH):
                t0 = ci * 4
                an_fm = sbf.tile([A_DIM, CH], BFD, tag="anf", name=tag + "anf")
                ar_tiles = []
                for i in range(4):
                    ar = sbq.tile([P, A_DIM], FD, tag="ar", name=tag + "ar")
                    nc.sync.dma_start(
                        out=ar[:], in_=t_angle_res[(t0 + i) * P:(t0 + i + 1) * P, :])
                    anm = rownorm(ar[:], A_DIM, tag + "an")
                    ar_tiles.append(ar)
                    transpose_to(an_fm[:, i * 128:(i + 1) * 128], anm[:], A_DIM)
                node_fm = sbf.tile([N_DIM, CH], BFD, tag="nf", name=tag + "nf")
                eij_fm = sbf.tile([N_DIM, CH], BFD, tag="jf", name=tag + "jf")
                eik_fm = sbf.tile([N_DIM, CH], BFD, tag="kf", name=tag + "kf")
                envs = []
                for i in range(4):
                    gn = sbg.tile([P, N_DIM], BFD, tag="gn", name=tag + "gn")
                    gather(gn[:], (t_nupd_full if pass3 else t_node_tbl)[:, :],
                           meta["a_n2a"], t0 + i)
                    transpose_to(node_fm[:, i * 128:(i + 1) * 128], gn[:], P)
                    if pass3:
                        gj = sbg.tile([P, 256], BFD, tag="gj2", name=tag + "gj")
                        gather(gj[:], t_tbl2_full[:, :], meta["a_eij"], t0 + i)
                        transpose_to(eij_fm[:, i * 128:(i + 1) * 128], gj[:, 0:128], P)
                        gk = sbg.tile([P, 256], BFD, tag="gk2", name=tag + "gk")
                        gather(gk[:], t_tbl2_full[:, :], meta["a_eik"], t0 + i)
                        transpose_to(eik_fm[:, i * 128:(i + 1) * 128], gk[:, 0:128], P)
                        env = sbq.tile([P, P], BFD, tag="env", name=tag + "env")
                        nc.vector.tensor_tensor(out=env[:], in0=gj[:, 128:256],
                                                in1=gk[:, 128:256], op=OP.mult)
                        envs.append(env)
                    else:
                        gj = sbg.tile([P, N_DIM], BFD, tag="gj1", name=tag + "gj")
                        gather(gj[:], t_tbl1_full[:, :], meta["a_eij"], t0 + i)
                        transpose_to(eij_fm[:, i * 128:(i + 1) * 128], gj[:], P)
                        gk = sbg.tile([P, N_DIM], BFD, tag="gk1", name=tag + "gk")
                        gather(gk[:], t_tbl1_full[:, :], meta["a_eik"], t0 + i)
                        transpose_to(eik_fm[:, i * 128:(i + 1) * 128], gk[:], P)
                rhs = [an_fm[:], node_fm[:], eij_fm[:], eik_fm[:]]
                e_fm = None
                if not pass3:
                    fps = PS([P, CH], "f")
                    gts = wsb["g1"]
                    for ki in range(4):
                        nc.tensor.matmul(fps[:], lhsT=gts[ki][:], rhs=rhs[ki],
                                         start=(ki == 0), stop=(ki == 3))
                    e_fm = sb.tile([P, CH], BFD, tag="efm", name=tag + "ef")
                    nc.scalar.activation(e_fm[:], fps[:], AF.Exp)
                msgs = gmlp_chunk(rhs, w_in, w_out,
                                  None if pass3 else meta["a_sw"], t0, tag)
                for i in range(4):
                    t = t0 + i
                    if not pass3:
                        ep = psp.tile([P, P], BFD, tag="tp", name="ps_tp2", space="PSUM")
                        nc.tensor.transpose(ep[:], e_fm[:, i * 128:(i + 1) * 128],
                                            ident[:])
                        et = sb.tile([P, 256], BFD, tag="et", name=tag + "et")
                        nc.scalar.copy(out=et[:, 0:128], in_=ep[:])
                        nc.vector.tensor_tensor(out=et[:, 128:256], in0=et[:, 0:128],
                                                in1=msgs[i][:], op=OP.mult)
                        src = et
                        width = 256
                    else:
                        t3a = sb.tile([P, 128], BFD, tag="t3a", name=tag + "t3a")
                        nc.vector.tensor_tensor(out=t3a[:], in0=msgs[i][:],
                                                in1=envs[i][:], op=OP.mult)
                        t3 = sb.tile([P, 128], BFD, tag="t3", name=tag + "t3")
                        nc.vector.tensor_scalar(out=t3[:], in0=t3a[:],
                                                scalar1=meta["a_sw"][:, t:t + 1],
                                                scalar2=None, op0=OP.mult)
                        src = t3
                        width = 128
                    sel = build_sel(meta["a_slot"], meta["a_first"], t)
                    scp = PS([P, 256], "misc")
                    nc.tensor.matmul(scp[:, :width], lhsT=sel[:], rhs=src[:],
                                     start=True, stop=True)
                    scb = sb.tile([P, width], BFD, tag="scb" + str(width),
                                  name=tag + "scb")
                    nc.scalar.copy(out=scb[:], in_=scp[:, :width])
                    scatter_tile(scb[:], meta["a_scat"], t, t_accA, t_accB)
                    if pass3:
                        auf = sb.tile([P, P], BFD, tag="auf", name=tag + "auf")
                        transpose_to(auf[:], msgs[i][:], P)
                        aop = PS([P, A_DIM], "misc")
                        nc.tensor.matmul(aop[:], lhsT=auf[:], rhs=wsb["w3_aproj"][0][:],
                                         start=True, stop=True)
                        aout = sb.tile([P, A_DIM], FD, tag="aout", name=tag + "aout")
                        nc.vector.tensor_tensor(out=aout[:], in0=aop[:],
                                                in1=ar_tiles[i][:], op=OP.add)
                        nc.sync.dma_start(out=t_angle_out[t * P:(t + 1) * P, :],
                                          in_=aout[:])

        angle_pass(pass3=False)

        # ================= consume edge blocks =================
        def consume_edge_blocks(dst_t, src_res_t, pass3):
            for b in range(NT_E):
                ga = sb.tile([P, 256], BFD, tag="cba", name="cba")
                nc.sync.dma_start(out=ga[:], in_=t_accA[b * P:(b + 1) * P, :])
                gb = sb.tile([P, 256], BFD, tag="cbb", name="cbb")
                nc.sync.dma_start(out=gb[:], in_=t_accB[b * P:(b + 1) * P, :])
                accf = sb.tile([P, 256], FD, tag="cbf", name="cbf")
                nc.vector.tensor_tensor(out=accf[:], in0=ga[:], in1=gb[:], op=OP.add)
                res = sb.tile([P, N_DIM], FD, tag="cbr", name="cbr")
                nc.sync.dma_start(out=res[:], in_=src_res_t[b * P:(b + 1) * P, :])
                if not pass3:
                    s_eps = sb.tile([P, 128], FD, tag="cbs", name="cbs")
                    nc.vector.tensor_scalar(out=s_eps[:], in0=accf[:, 0:128],
                                            scalar1=1e-12, scalar2=None, op0=OP.add)
                    srec = sb.tile([P, 128], FD, tag="cbrc", name="cbrc")
                    nc.vector.reciprocal(srec[:], s_eps[:])
                    delta = sb.tile([P, 128], FD, tag="cbd", name="cbd")
                    nc.vector.tensor_tensor(out=delta[:], in0=accf[:, 128:256],
                                            in1=srec[:], op=OP.mult)
                else:
                    dbf = sb.tile([P, 128], BFD, tag="cdb", name="cdb")
                    nc.vector.tensor_copy(out=dbf[:], in_=accf[:, 0:128])
                    dfm = sb.tile([P, 128], BFD, tag="cdf", name="cdf")
                    transpose_to(dfm[:], dbf[:], P)
                    pp = PS([P, 128], "misc")
                    nc.tensor.matmul(pp[:], lhsT=dfm[:], rhs=wsb["w3_eproj"][0][:],
                                     start=True, stop=True)
                    delta = sb.tile([P, 128], FD, tag="cbd", name="cbd")
                    nc.vector.tensor_scalar(out=delta[:], in0=pp[:], scalar1=A_SCALE,
                                            scalar2=None, op0=OP.mult)
                out = sb.tile([P, N_DIM], FD, tag="cbo", name="cbo")
                nc.vector.tensor_tensor(out=out[:], in0=res[:], in1=delta[:],
                                        op=OP.add)
                nc.sync.dma_start(out=dst_t[b * P:(b + 1) * P, :], in_=out[:])

        consume_edge_blocks(t_eres1, t_edge_res, pass3=False)

        # ================= node tables =================
        def make_node_table(src_t, dst_t):
            for b in range(NBLK_N):
                nr = sb.tile([P, N_DIM], FD, tag="ntr", name="ntr")
                if b * P < npc:
                    nc.sync.dma_start(out=nr[:], in_=src_t[b * P:(b + 1) * P, :])
                else:
                    nc.vector.memset(nr[:], 0.0)
                nm = rownorm(nr[:], N_DIM, "nt")
                nc.sync.dma_start(out=dst_t[b * P:(b + 1) * P, :], in_=nm[:])

        make_node_table(t_node_res, t_nodeN)

        # ================= edge passes =================
        def edge_pass(pass4):
            tag = "e4" if pass4 else "e2"
            w_in = wsb["w4_in"] if pass4 else wsb["w2a_in"]
            w_out = [wsb["w4_out"][0][:], wsb["w4_out"][1][:]] if pass4 else \
                    [wsb["w2a_out"][0][:], wsb["w2a_out"][1][:]]
            src_res = t_eres3 if pass4 else t_eres1
            node_src = t_nodeN4 if pass4 else t_nodeN
            for ci in range(E_max // CH):
                t0 = ci * 4
                rbf_ch = sbg.tile([R_DIM, CH], BFD, tag="rbfc", name=tag + "rbf")
                nc.sync.dma_start(out=rbf_ch[:], in_=t_rbf_t[:, ci * CH:(ci + 1) * CH])
                en_fm = sbf.tile([N_DIM, CH], BFD, tag="ef", name=tag + "ef")
                res_tiles = []
                for i in range(4):
                    er = sbq.tile([P, N_DIM], FD, tag="er", name=tag + "er")
                    nc.sync.dma_start(
                        out=er[:], in_=src_res[(t0 + i) * P:(t0 + i + 1) * P, :])
                    enm = rownorm(er[:], N_DIM, tag + "en")
                    res_tiles.append(er)
                    transpose_to(en_fm[:, i * 128:(i + 1) * 128], enm[:], P)
                node_fm = sbf.tile([N_DIM, CH], BFD, tag="nf", name=tag + "nf")
                ext_fm = sbf.tile([N_DIM, CH], BFD, tag="xf", name=tag + "xf")
                for i in range(4):
                    gn = sbg.tile([P, N_DIM], BFD, tag="gn", name=tag + "gn")
                    gather(gn[:], node_src[:, :], meta["e_gnode"], t0 + i)
                    transpose_to(node_fm[:, i * 128:(i + 1) * 128], gn[:], P)
                    gx = sbg.tile([P, N_DIM], BFD, tag="gx", name=tag + "gx")
                    gather(gx[:], t_ext_tbl[:, :], meta["e_gext"], t0 + i)
                    transpose_to(ext_fm[:, i * 128:(i + 1) * 128], gx[:], P)
                rhs = [en_fm[:], node_fm[:], ext_fm[:]]
                e_fm = None
                if not pass4:
                    fps = PS([P, CH], "f")
                    nc.tensor.matmul(fps[:], lhsT=wsb["g2"][0][:], rhs=en_fm[:],
                                     start=True, stop=True)
                    e_fm = sb.tile([P, CH], BFD, tag="efm", name=tag + "efm")
                    nc.scalar.activation(e_fm[:], fps[:], AF.Exp)
                msgs = gmlp_chunk(rhs, w_in, w_out,
                                  None if pass4 else meta["e_sw"], t0, tag)
                for i in range(4):
                    t = t0 + i
                    if not pass4:
                        ep = psp.tile([P, P], BFD, tag="tp", name="ps_tp2", space="PSUM")
                        nc.tensor.transpose(ep[:], e_fm[:, i * 128:(i + 1) * 128],
                                            ident[:])
                        et = sb.tile([P, 256], BFD, tag="et", name=tag + "et")
                        nc.scalar.copy(out=et[:, 0:128], in_=ep[:])
                        nc.vector.tensor_tensor(out=et[:, 128:256], in0=et[:, 0:128],
                                                in1=msgs[i][:], op=OP.mult)
                        src = et
                        width = 256
                    else:
                        evp = PS([P, P], "misc")
                        nc.tensor.matmul(evp[:], lhsT=rbf_ch[:, i * P:(i + 1) * P],
                                         rhs=wsb["w4_env"][0][:], start=True, stop=True)
                        ev = sb.tile([P, P], BFD, tag="evs", name=tag + "evs")
                        nc.scalar.activation(ev[:], evp[:], AF.Sigmoid)
                        t4a = sb.tile([P, 128], BFD, tag="t4a", name=tag + "t4a")
                        nc.vector.tensor_tensor(out=t4a[:], in0=msgs[i][:],
                                                in1=ev[:], op=OP.mult)
                        t4 = sb.tile([P, 128], BFD, tag="t4", name=tag + "t4")
                        nc.vector.tensor_scalar(out=t4[:], in0=t4a[:],
                                                scalar1=meta["e_sw"][:, t:t + 1],
                                                scalar2=None, op0=OP.mult)
                        src = t4
                        width = 128
                    sel = build_sel(meta["e_slot"], meta["e_first"], t)
                    scp = PS([P, 256], "misc")
                    nc.tensor.matmul(scp[:, :width], lhsT=sel[:], rhs=src[:],
                                     start=True, stop=True)
                    scb = sb.tile([P, width], BFD, tag="scb" + str(width),
                                  name=tag + "scb")
                    nc.scalar.copy(out=scb[:], in_=scp[:, :width])
                    scatter_tile(scb[:], meta["e_scat"], t, t_naccA, t_naccB)
                    if pass4:
                        nf2 = sb.tile([P, P], BFD, tag="nf2", name=tag + "nf2")
                        transpose_to(nf2[:], msgs[i][:], P)
                        pp = PS([P, 128], "misc")
                        nc.tensor.matmul(pp[:], lhsT=nf2[:], rhs=wsb["w4_eproj"][0][:],
                                         start=True, stop=True)
                        eo = sb.tile([P, N_DIM], FD, tag="eo", name=tag + "eo")
                        nc.vector.tensor_tensor(out=eo[:], in0=res_tiles[i][:],
                                                in1=pp[:], op=OP.add)
                        nc.sync.dma_start(out=t_edge_out[t * P:(t + 1) * P, :],
                                          in_=eo[:])
                if not pass4:
                    msgs_e = gmlp_chunk(rhs, wsb["w2e_in"],
                                        [wsb["w2e_out"][0][:], wsb["w2e_out"][1][:]],
                                        meta["e_sw"], t0, tag + "b")
                    for i in range(4):
                        t = t0 + i
                        r2 = sb.tile([P, N_DIM], FD, tag="r2", name=tag + "r2")
                        nc.vector.tensor_tensor(out=r2[:], in0=res_tiles[i][:],
                                                in1=msgs_e[i][:], op=OP.add)
                        nc.sync.dma_start(out=t_eres2[t * P:(t + 1) * P, :], in_=r2[:])
                        nm2 = rownorm(r2[:], N_DIM, tag + "n2")
                        nc.sync.dma_start(out=t_tbl2_own[t * P:(t + 1) * P, 0:128],
                                          in_=nm2[:])
                        evp = PS([P, P], "misc")
                        nc.tensor.matmul(evp[:], lhsT=rbf_ch[:, i * P:(i + 1) * P],
                                         rhs=wsb["w3_env"][0][:], start=True, stop=True)
                        ev = sb.tile([P, P], BFD, tag="ev3", name=tag + "ev3")
                        nc.scalar.activation(ev[:], evp[:], AF.Sigmoid)
                        nc.sync.dma_start(out=t_tbl2_own[t * P:(t + 1) * P, 128:256],
                                          in_=ev[:])

        edge_pass(pass4=False)

        # ================= consume node blocks =================
        def consume_node_blocks(dst_t, src_t, pass4):
            for b in range(npc // P):
                ga = sb.tile([P, 256], BFD, tag="nba", name="nba")
                nc.sync.dma_start(out=ga[:], in_=t_naccA[b * P:(b + 1) * P, :])
                gb = sb.tile([P, 256], BFD, tag="nbb", name="nbb")
                nc.sync.dma_start(out=gb[:], in_=t_naccB[b * P:(b + 1) * P, :])
                accf = sb.tile([P, 256], FD, tag="nbf", name="nbf")
                nc.vector.tensor_tensor(out=accf[:], in0=ga[:], in1=gb[:], op=OP.add)
                res = sb.tile([P, N_DIM], FD, tag="nbr", name="nbr")
                nc.sync.dma_start(out=res[:], in_=src_t[b * P:(b + 1) * P, :])
                if not pass4:
                    s_eps = sb.tile([P, 128], FD, tag="nbs", name="nbs")
                    nc.vector.tensor_scalar(out=s_eps[:], in0=accf[:, 0:128],
                                            scalar1=1e-12, scalar2=None, op0=OP.add)
                    srec = sb.tile([P, 128], FD, tag="nbrc", name="nbrc")
                    nc.vector.reciprocal(srec[:], s_eps[:])
                    delta = sb.tile([P, 128], FD, tag="nbd", name="nbd")
                    nc.vector.tensor_tensor(out=delta[:], in0=accf[:, 128:256],
                                            in1=srec[:], op=OP.mult)
                else:
                    dbf = sb.tile([P, 128], BFD, tag="ndb", name="ndb")
                    nc.vector.tensor_copy(out=dbf[:], in_=accf[:, 0:128])
                    dfm = sb.tile([P, 128], BFD, tag="ndf", name="ndf")
                    transpose_to(dfm[:], dbf[:], P)
                    pp = PS([P, 128], "misc")
                    nc.tensor.matmul(pp[:], lhsT=dfm[:], rhs=wsb["w4_nproj"][0][:],
                                     start=True, stop=True)
                    delta = sb.tile([P, 128], FD, tag="nbd", name="nbd")
                    nc.vector.tensor_scalar(out=delta[:], in0=pp[:], scalar1=E_SCALE,
                                            scalar2=None, op0=OP.mult)
                out = sb.tile([P, N_DIM], FD, tag="nbo", name="nbo")
                nc.vector.tensor_tensor(out=out[:], in0=res[:], in1=delta[:], op=OP.add)
                nc.sync.dma_start(out=dst_t[b * P:(b + 1) * P, :], in_=out[:])

        consume_node_blocks(t_nres1, t_node_res, pass4=False)

        for b in range(npc // P):
            nr = sb.tile([P, N_DIM], FD, tag="nu", name="nu")
            nc.sync.dma_start(out=nr[:], in_=t_nres1[b * P:(b + 1) * P, :])
            nb = sb.tile([P, N_DIM], BFD, tag="nub", name="nub")
            nc.vector.tensor_copy(out=nb[:], in_=nr[:])
            nc.sync.dma_start(out=t_nupd_own[b * P:(b + 1) * P, :], in_=nb[:])

        nc.gpsimd.collective_compute(
            "AllGather", OP.bypass,
            ins=[t_tbl2_own[:, :].opt()], outs=[t_tbl2_full[:, :].opt()],
            replica_groups=[list(range(NCORES))])
        nc.gpsimd.collective_compute(
            "AllGather", OP.bypass,
            ins=[t_nupd_own[:, :].opt()], outs=[t_nupd_full[:, :].opt()],
            replica_groups=[list(range(NCORES))])

        angle_pass(pass3=True)
        consume_edge_blocks(t_eres3, t_eres2, pass3=True)
        make_node_table(t_nres1, t_nodeN4)
        edge_pass(pass4=True)
        consume_node_blocks(t_node_out, t_nres1, pass4=True)

    nc.compile()
    return nc


# ----------------------------------------------------------------------------
# entry point
# ----------------------------------------------------------------------------

def kernel(_return_parts=False, **inputs):
    nframes = int(inputs["nframes"])
    nloc = int(inputs["nloc"])
    num_nodes = nframes * nloc
    node_ebd_ext = np.asarray(inputs["node_ebd_ext"], F32)
    edge_ebd = np.asarray(inputs["edge_ebd"], F32)
    angle_ebd = np.asarray(inputs["angle_ebd"], F32)
    sw = np.asarray(inputs["sw"], F32)
    a_sw = np.asarray(inputs["a_sw"], F32)
    edge_index = np.asarray(inputs["edge_index"], np.int64)
    angle_index = np.asarray(inputs["angle_index"], np.int64)
    edge_rbf = np.asarray(inputs["edge_rbf"], F32)

    n2e, n_ext2e = edge_index[0], edge_index[1]
    n2a, eij2a, eik2a = angle_index[0], angle_index[1], angle_index[2]
    node_ebd = node_ebd_ext[:, :nloc, :].reshape(-1, N_DIM)
    node_ext = node_ebd_ext.reshape(-1, N_DIM)

    plan = build_plan(n2e, n_ext2e, n2a, eij2a, eik2a, num_nodes)
    npc, E_max, A_max = plan["npc"], plan["E_max"], plan["A_max"]
    NT_A, NT_E = A_max // P, E_max // P
    NODE_TBL = npc + P
    ETBL = E_max + P

    W = fold_weights(inputs["params"])

    nc = build_kernel(E_max, A_max, npc)

    IOTA = np.broadcast_to(np.arange(P, dtype=F32)[None, :], (P, P)).copy()
    ONES = np.ones((P, 1), BF)
    ONE1 = np.ones((1, 1), F32)
    node_tbl = node_ebd.astype(BF)
    ext_tbl = node_ext.astype(BF)

    in_maps = []
    for c in range(NCORES):
        eids = plan["edge_lists"][c]
        ke = len(eids)
        s2, cols = plan["per_core_a"][c]
        av = s2 >= 0

        edge_res = np.zeros((E_max, N_DIM), F32)
        edge_res[:ke] = edge_ebd[eids]
        angle_res = np.zeros((A_max, A_DIM), F32)
        angle_res[av] = angle_ebd[cols["aid"][av]]
        node_res = node_ebd[c * npc:(c + 1) * npc].astype(F32)
        a_sw_arr = np.zeros(A_max, F32)
        a_sw_arr[av] = a_sw[cols["aid"][av]]
        sw_arr = np.zeros(E_max, F32)
        sw_arr[:ke] = sw[eids]
        rbf_t = np.zeros((R_DIM, E_max), BF)
        rbf_t[:, :ke] = edge_rbf[eids].T.astype(BF)

        n2a_idx = np.where(av, cols["n2a"], 0).astype(np.int64)
        eik_g = np.where(av, cols["eik_core"] * E_max + cols["eik_slot"], 0)
        eij_g = np.where(av, c * E_max + cols["eij_slot_g"], 0)
        a_slot = s2.astype(F32)
        a_first, a_scat = _scatter_plan(s2, NT_A, E_max)  # trash row = E_max..E_max+127
        a_first_b = np.broadcast_to(a_first[None, :], (P, NT_A)).astype(F32)

        n2e_loc = np.full(E_max, -1.0, F32)
        n2e_loc[:ke] = (n2e[eids] - c * npc).astype(F32)
        e_slot_i = np.full(E_max, -1, np.int64)
        e_slot_i[:ke] = n2e[eids] - c * npc
        gnode_idx = np.where(e_slot_i >= 0, e_slot_i, 0)
        gext_idx = np.zeros(E_max, np.int64)
        gext_idx[:ke] = n_ext2e[eids]
        e_first, e_scat = _scatter_plan(e_slot_i, NT_E, NODE_TBL)
        e_first_b = np.broadcast_to(e_first[None, :], (P, NT_E)).astype(F32)

        im = {
            "angle_res": angle_res, "edge_res": edge_res, "node_res": node_res,
            "node_tbl": node_tbl, "ext_tbl": ext_tbl, "rbf_t": rbf_t,
            "a_n2a": _blocked_i32(n2a_idx, NT_A),
            "a_eik": _blocked_i32(eik_g, NT_A),
            "a_eij": _blocked_i32(eij_g, NT_A),
            "a_slot": _blocked_f32(a_slot, NT_A),
            "a_first": np.ascontiguousarray(a_first_b),
            "a_scat": _blocked_i32(a_scat, NT_A),
            "a_sw_b": _blocked_f32(a_sw_arr, NT_A),
            "e_gnode": _blocked_i32(gnode_idx, NT_E),
            "e_gext": _blocked_i32(gext_idx, NT_E),
            "e_slot": _blocked_f32(n2e_loc, NT_E),
            "e_first": np.ascontiguousarray(e_first_b),
            "e_scat": _blocked_i32(e_scat, NT_E),
            "e_sw_b": _blocked_f32(sw_arr, NT_E),
            "iota": IOTA, "ones_col": ONES, "one_one": ONE1,
            "accA": np.zeros((ETBL, 256), BF),
            "accB": np.zeros((ETBL, 256), BF),
            "naccA": np.zeros((NODE_TBL + P, 256), BF),
            "naccB": np.zeros((NODE_TBL + P, 256), BF),
        }
        for k, v in W.items():
            im["W_" + k] = v
        in_maps.append(im)

    def stitch(results):
        node_out = np.zeros((num_nodes, N_DIM), F32)
        edge_out = np.zeros_like(edge_ebd)
        angle_out = np.zeros_like(angle_ebd)
        for c in range(NCORES):
            r = results[c]
            eids = plan["edge_lists"][c]
            s2, cols = plan["per_core_a"][c]
            av = s2 >= 0
            node_out[c * npc:(c + 1) * npc] = r["node_out"]
            edge_out[eids] = r["edge_out"][:len(eids)]
            angle_out[cols["aid"][av]] = r["angle_out"][av]
        return (node_out.reshape(nframes, nloc, N_DIM), edge_out, angle_out)

    if _return_parts:
        return nc, in_maps, stitch
    res = bass_utils.run_bass_kernel_spmd(nc, in_maps, core_ids=list(range(NCORES)))
    return stitch(res.results)


# revision 13
# speedup vs baseline: 3.9236x; 3.9236x over previous
"""DPA3 layer (nn_DPA3NextLayer) on 8 Trainium2 NeuronCores via Bass/Tile.

Strategy:
- Shard nodes into 8 contiguous blocks; edges go to the core owning n2e;
  angles go to the core owning eij2a. All segment reductions are core-local.
- Edges sorted by n2e, angles sorted by local slot of eij2a -> segment sums
  are computed per 128-item tile with a one-hot (is_equal) matrix + PE matmul,
  then written to per-parity DRAM partial tables with an indirect scatter
  (disjoint row ranges per parity -> race-free, no RMW).
- The dimwise softmax needs no segment max (values are small; stabilizer
  cancels up to the 1e-12 eps) and no gather-back: seg(e*msg)/(seg(e)+eps).
- Gathers (node[n2a], edge[eik2a], ...) via indirect DMA from replicated /
  allgathered bf16 tables. Two AllGathers: normalized original edges (for
  sub-block 1) and [normalized updated edges | envelope] + updated nodes
  (between sub-blocks 2 and 3).
- Residual stream fp32; matmuls and gathered operands bf16.
"""
import math
import numpy as np
import ml_dtypes

import sys
if "/opt/trn_rl_repo" not in sys.path:
    sys.path.insert(0, "/opt/trn_rl_repo")

import concourse.bass as bass
import concourse.mybir as mybir
import concourse.tile as tile
from concourse import bacc
from concourse import bass_utils
from concourse.masks import make_identity

P = 128
N_DIM, E_DIM, A_DIM, R_DIM = 128, 128, 64, 12
NCORES = 8
CH = 512  # items per chunk
A_SEL, NNEI, SEL_RF = 40, 120, 10.0
A_SCALE = float((A_SEL / SEL_RF) ** -0.5)
E_SCALE = float((NNEI / SEL_RF) ** -0.5)
BF = ml_dtypes.bfloat16
F32 = np.float32


# ----------------------------------------------------------------------------
# host planning
# ----------------------------------------------------------------------------

def _blocked_i32(x, nt):
    # [P*nt] -> [P, nt] with column t = items of tile t
    return np.ascontiguousarray(x.reshape(nt, P).T).astype(np.int32)


def _blocked_f32(x, nt):
    return np.ascontiguousarray(x.reshape(nt, P).T).astype(np.float32)


def _insert_pads_span(slots, other_cols, chunk):
    """Insert pad items (slot=-1) so every 128-item tile spans <=128 slots.
    slots: [n] nondecreasing (real items). other_cols: dict name->array[n].
    Returns padded arrays (multiple of chunk)."""
    n = slots.shape[0]
    out_slots = []
    outs = {k: [] for k in other_cols}
    i = 0
    cur = []  # current tile slots
    def flushable(s):
        return (not cur) or (s - cur[0] < P)
    while i < n:
        s = slots[i]
        if flushable(s):
            cur.append(s)
            out_slots.append(s)
            for k, v in other_cols.items():
                outs[k].append(v[i])
            i += 1
            if len(cur) == P:
                cur = []
        else:
            # pad until tile boundary
            out_slots.append(-1)
            for k, v in other_cols.items():
                outs[k].append(0)
            cur.append(cur[0])  # dummy to count fill
            if len(cur) == P:
                cur = []
    slots2 = np.array(out_slots, np.int64)
    res = {k: np.array(vs) for k, vs in outs.items()}
    # pad tail to chunk multiple
    total = int(math.ceil(len(slots2) / chunk) * chunk)
    pad = total - len(slots2)
    slots2 = np.concatenate([slots2, np.full(pad, -1, np.int64)])
    for k in res:
        res[k] = np.concatenate([res[k], np.zeros(pad, res[k].dtype)])
    return slots2, res


def build_plan(n2e, n_ext2e, n2a, eij2a, eik2a, num_nodes):
    npc = num_nodes // NCORES
    n_edge = n2e.shape[0]
    core_of_edge = (n2e // npc).astype(np.int64)
    order_e = np.argsort(n2e, kind="stable")
    counts_e = np.bincount(core_of_edge, minlength=NCORES)

    slot_of_edge = np.empty(n_edge, np.int64)
    edge_lists = []
    pos = 0
    for c in range(NCORES):
        k = int(counts_e[c])
        ids = order_e[pos:pos + k]
        edge_lists.append(ids)
        slot_of_edge[ids] = np.arange(k)
        pos += k

    core_of_angle = core_of_edge[eij2a]
    akey = slot_of_edge[eij2a]
    order_a = np.lexsort((akey, core_of_angle))
    counts_a = np.bincount(core_of_angle, minlength=NCORES)

    # Per-core angle arrays with span-limiting pads
    per_core_a = []
    pos = 0
    max_alen = 0
    for c in range(NCORES):
        k = int(counts_a[c])
        aids = order_a[pos:pos + k]
        pos += k
        slots = slot_of_edge[eij2a[aids]]
        cols = {
            "aid": aids,
            "n2a": n2a[aids],
            "eik_core": core_of_edge[eik2a[aids]],
            "eik_slot": slot_of_edge[eik2a[aids]],
            "eij_slot_g": slot_of_edge[eij2a[aids]],
        }
        s2, cols2 = _insert_pads_span(slots, cols, CH)
        cols2["valid"] = np.concatenate([
            np.zeros(0, bool),
            (s2 >= 0)])
        per_core_a.append((s2, cols2))
        max_alen = max(max_alen, len(s2))
    A_max = int(math.ceil(max_alen / CH) * CH)
    # re-pad all to A_max
    for c in range(NCORES):
        s2, cols2 = per_core_a[c]
        pad = A_max - len(s2)
        s2 = np.concatenate([s2, np.full(pad, -1, np.int64)])
        for k in cols2:
            cols2[k] = np.concatenate([cols2[k], np.zeros(pad, cols2[k].dtype)])
        per_core_a[c] = (s2, cols2)

    E_max = int(math.ceil(counts_e.max() / CH) * CH)
    return dict(npc=npc, E_max=E_max, A_max=A_max, counts_e=counts_e,
                counts_a=counts_a, edge_lists=edge_lists,
                slot_of_edge=slot_of_edge, core_of_edge=core_of_edge,
                per_core_a=per_core_a)


def _scatter_plan(slots, n_tiles, trash_row):
    """Per-tile first slot + per-item scatter target rows with parity tables.
    Returns first[P?]: [n_tiles] int, scat rows [n_tiles*P] int32 (within-table
    row), parity handled by caller (tile index % 2). Rows outside
    [first_t, first_next_same_parity) -> trash."""
    BIG = 10 ** 9
    first = np.zeros(n_tiles, np.int64)
    has_real = np.zeros(n_tiles, bool)
    for t in range(n_tiles):
        ts = slots[t * P:(t + 1) * P]
        real = ts[ts >= 0]
        has_real[t] = len(real) > 0
        first[t] = real[0] if len(real) else BIG
    scat = np.zeros(n_tiles * P, np.int64)
    for t in range(n_tiles):
        if not has_real[t]:
            scat[t * P:(t + 1) * P] = trash_row
            continue
        lim = first[t + 2] if t + 2 < n_tiles else BIG
        for p in range(P):
            r = first[t] + p
            scat[t * P + p] = r if r < lim else trash_row
    first = np.where(has_real, first, 0)
    return first, scat


# ----------------------------------------------------------------------------
# weights (folded, bf16)
# ----------------------------------------------------------------------------

def fold_weights(params):
    p = {k: (np.asarray(v, F32) if not isinstance(v, dict) else
             {kk: np.asarray(vv, F32) for kk, vv in v.items()}) for k, v in params.items()}
    ones_n = np.ones(N_DIM, F32)
    W = {}

    def gmlp(g, fold):
        w_in = g["w_in"] * fold[:, None]
        w_out = g["w_out"] * g["norm"][:, None]
        return w_in.astype(BF), w_out.astype(BF)

    f1 = np.concatenate([p["line_attn_norm_a"], ones_n,
                         p["line_attn_norm_e"], p["line_attn_norm_e"]])
    W["w1_in"], W["w1_out"] = gmlp(p["line_attn_mlp"], f1)
    W["g1"] = (p["line_attn_gate"] * f1[:, None]).astype(BF)

    f2 = np.concatenate([p["atom_attn_norm_e"], p["atom_attn_norm_n"], ones_n])
    W["w2a_in"], W["w2a_out"] = gmlp(p["atom_attn_mlp"], f2)
    W["w2e_in"], W["w2e_out"] = gmlp(p["atom_attn_edge_mlp"], f2)
    W["g2"] = (p["atom_attn_src_gate"] * p["atom_attn_norm_e"][:, None]).astype(BF)

    f3 = np.concatenate([p["line_ref_norm_a"], ones_n,
                         p["line_ref_norm_e"], p["line_ref_norm_e"]])
    W["w3_in"], W["w3_out"] = gmlp(p["line_ref_mlp"], f3)
    W["w3_env"] = p["line_ref_env"].astype(BF)
    W["w3_eproj"] = p["line_ref_edge_proj"].astype(BF)
    W["w3_aproj"] = p["line_ref_angle_proj"].astype(BF)

    f4 = np.concatenate([p["atom_ref_norm_e"], p["atom_ref_norm_n"], ones_n])
    W["w4_in"], W["w4_out"] = gmlp(p["atom_ref_mlp"], f4)
    W["w4_env"] = p["atom_ref_env"].astype(BF)
    W["w4_nproj"] = p["atom_ref_node_proj"].astype(BF)
    W["w4_eproj"] = p["atom_ref_edge_proj"].astype(BF)
    return W


# ----------------------------------------------------------------------------
# bass kernel builder
# ----------------------------------------------------------------------------

BFD = mybir.dt.bfloat16
FD = mybir.dt.float32
ID = mybir.dt.int32
AF = mybir.ActivationFunctionType
OP = mybir.AluOpType



def build_kernel(E_max, A_max, npc, debug_outputs=False):
    NT_A = A_max // P
    NT_E = E_max // P
    NODE_TBL = npc + P
    NBLK_N = NODE_TBL // P
    ETBL = E_max + P

    nc = bacc.Bacc(None)

    def inp(name, shape, dt):
        return nc.dram_tensor(name, list(shape), dt, kind="ExternalInput")

    def outp(name, shape, dt):
        return nc.dram_tensor(name, list(shape), dt, kind="ExternalOutput")

    t_angle_res = inp("angle_res", [A_max, A_DIM], FD)
    t_edge_res = inp("edge_res", [E_max, N_DIM], FD)
    t_node_res = inp("node_res", [npc, N_DIM], FD)
    t_node_tbl = inp("node_tbl", [NCORES * npc, N_DIM], BFD)
    t_ext_tbl = inp("ext_tbl", [12288, N_DIM], BFD)
    t_rbf_t = inp("rbf_t", [R_DIM, E_max], BFD)

    t_a_n2a = inp("a_n2a", [P, NT_A], ID)
    t_a_eik = inp("a_eik", [P, NT_A], ID)
    t_a_eij = inp("a_eij", [P, NT_A], ID)
    t_a_slot = inp("a_slot", [P, NT_A], FD)
    t_a_first = inp("a_first", [P, NT_A], FD)
    t_a_scat = inp("a_scat", [P, NT_A], ID)
    t_a_sw = inp("a_sw_b", [P, NT_A], FD)

    t_e_gnode = inp("e_gnode", [P, NT_E], ID)
    t_e_gext = inp("e_gext", [P, NT_E], ID)
    t_e_slot = inp("e_slot", [P, NT_E], FD)
    t_e_first = inp("e_first", [P, NT_E], FD)
    t_e_scat = inp("e_scat", [P, NT_E], ID)
    t_e_sw = inp("e_sw_b", [P, NT_E], FD)

    t_iota = inp("iota", [P, P], FD)
    t_ones = inp("ones_col", [P, 1], BFD)
    t_one1 = inp("one_one", [1, 1], FD)

    wshapes = dict(
        w1_in=(448, 512), w1_out=(256, 128), g1=(448, 128),
        w2a_in=(384, 512), w2a_out=(256, 128), w2e_in=(384, 512), w2e_out=(256, 128),
        g2=(128, 128),
        w3_in=(448, 512), w3_out=(256, 128), w3_env=(12, 128), w3_eproj=(128, 128),
        w3_aproj=(128, 64),
        w4_in=(384, 512), w4_out=(256, 128), w4_env=(12, 128), w4_nproj=(128, 128),
        w4_eproj=(128, 128),
    )
    t_w = {k: inp("W_" + k, v, BFD) for k, v in wshapes.items()}

    t_accA = inp("accA", [ETBL, 256], BFD)
    t_accB = inp("accB", [ETBL, 256], BFD)
    t_naccA = inp("naccA", [NODE_TBL + P, 256], BFD)
    t_naccB = inp("naccB", [NODE_TBL + P, 256], BFD)

    t_node_out = outp("node_out", [npc, N_DIM], FD)
    t_edge_out = outp("edge_out", [E_max, N_DIM], FD)
    t_angle_out = outp("angle_out", [A_max, A_DIM], FD)

    t_tbl1_own = nc.dram_tensor("tbl1_own", [E_max, N_DIM], BFD)
    t_tbl1_full = nc.dram_tensor("tbl1_full", [NCORES * E_max, N_DIM], BFD,
                                 addr_space="Shared")
    t_tbl2_own = nc.dram_tensor("tbl2_own", [E_max, 256], BFD)
    t_tbl2_full = nc.dram_tensor("tbl2_full", [NCORES * E_max, 256], BFD,
                                 addr_space="Shared")
    t_nupd_own = nc.dram_tensor("nupd_own", [npc, N_DIM], BFD)
    t_nupd_full = nc.dram_tensor("nupd_full", [NCORES * npc, N_DIM], BFD,
                                 addr_space="Shared")
    t_eres1 = nc.dram_tensor("eres1", [E_max, N_DIM], FD)
    t_eres2 = nc.dram_tensor("eres2", [E_max, N_DIM], FD)
    t_eres3 = nc.dram_tensor("eres3", [E_max, N_DIM], FD)
    t_nres1 = nc.dram_tensor("nres1", [npc, N_DIM], FD)
    t_nodeN = nc.dram_tensor("nodeN", [NODE_TBL, N_DIM], BFD)
    t_nodeN4 = nc.dram_tensor("nodeN4", [NODE_TBL, N_DIM], BFD)

    from contextlib import ExitStack
    _es = ExitStack()
    with tile.TileContext(nc) as tc, _es:
        sbc = _es.enter_context(tc.tile_pool(name="const", bufs=1))
        sbw = _es.enter_context(tc.tile_pool(name="wts", bufs=1))
        sb = _es.enter_context(tc.tile_pool(name="work", bufs=3))
        sbq = _es.enter_context(tc.tile_pool(name="quad", bufs=5))
        sbg = _es.enter_context(tc.tile_pool(name="gath", bufs=4))
        sbf = _es.enter_context(tc.tile_pool(name="fm", bufs=2))
        psp = _es.enter_context(tc.tile_pool(name="psp", bufs=1, space="PSUM"))

        def PS(shape, tag):
            return psp.tile(shape, FD, tag=tag, name="ps_" + tag, space="PSUM")

        iota_sb = sbc.tile([P, P], FD, name="iota_sb")
        nc.sync.dma_start(out=iota_sb[:], in_=t_iota[:, :])
        ident = sbc.tile([P, P], BFD, name="ident")
        make_identity(nc, ident[:])
        ones_sb = sbc.tile([P, 1], BFD, name="ones_sb")
        nc.sync.dma_start(out=ones_sb[:], in_=t_ones[:, :])
        one1_sb = sbc.tile([1, 1], FD, name="one1_sb")
        nc.sync.dma_start(out=one1_sb[:], in_=t_one1[:, :])

        meta = {}
        for nm, t, dt, w in [("a_n2a", t_a_n2a, ID, NT_A), ("a_eik", t_a_eik, ID, NT_A),
                             ("a_eij", t_a_eij, ID, NT_A), ("a_scat", t_a_scat, ID, NT_A),
                             ("a_slot", t_a_slot, FD, NT_A), ("a_first", t_a_first, FD, NT_A),
                             ("a_sw", t_a_sw, FD, NT_A),
                             ("e_gnode", t_e_gnode, ID, NT_E), ("e_gext", t_e_gext, ID, NT_E),
                             ("e_scat", t_e_scat, ID, NT_E), ("e_slot", t_e_slot, FD, NT_E),
                             ("e_first", t_e_first, FD, NT_E), ("e_sw", t_e_sw, FD, NT_E)]:
            tl = sbc.tile([P, w], dt, name="m_" + nm)
            nc.sync.dma_start(out=tl[:], in_=t[:, :])
            meta[nm] = tl

        ksplits = {k: ([64, 128, 128, 128] if kk == 448 else None)
                   for k, (kk, mm) in wshapes.items()}
        wsb = {}
        for k, (kk, mm) in wshapes.items():
            tls = []
            off = 0
            split = ksplits[k]
            si = 0
            while off < kk:
                h = split[si] if split else min(128, kk - off)
                si += 1
                tl = sbw.tile([h, mm], BFD, name=f"w_{k}_{off}")
                nc.sync.dma_start(out=tl[:], in_=t_w[k][off:off + h, :])
                tls.append(tl)
                off += h
            wsb[k] = tls

        # ---------------- helpers ----------------
        def rownorm(x_f32, d, name):
            sq = sb.tile([P, d], FD, tag="rn_sq", name=name + "sq")
            nc.vector.tensor_tensor(out=sq[:], in0=x_f32, in1=x_f32, op=OP.mult)
            ssum = sb.tile([P, 1], FD, tag="rn_ss", name=name + "ss")
            nc.vector.reduce_sum(ssum[:], sq[:], axis=mybir.AxisListType.X)
            ms = sb.tile([P, 1], FD, tag="rn_ms", name=name + "ms")
            nc.vector.tensor_scalar(out=ms[:], in0=ssum[:], scalar1=1.0 / d,
                                    scalar2=1e-6, op0=OP.mult, op1=OP.add)
            rms = sb.tile([P, 1], FD, tag="rn_rm", name=name + "rm")
            nc.scalar.activation(rms[:], ms[:], AF.Sqrt)
            inv = sb.tile([P, 1], FD, tag="rn_iv", name=name + "iv")
            nc.vector.reciprocal(inv[:], rms[:])
            nrm = sb.tile([P, d], BFD, tag="rn_nm", name=name + "nm")
            nc.vector.tensor_scalar(out=nrm[:], in0=x_f32, scalar1=inv[:],
                                    scalar2=None, op0=OP.mult)
            return nrm

        _tp_ctr = [0]
        def transpose_to(dst_ap, src_ap, h):
            _tp_ctr[0] ^= 1
            tg = "tp" if _tp_ctr[0] else "tp2"
            pt = psp.tile([P, P], BFD, tag=tg, name="ps_" + tg, space="PSUM")
            nc.tensor.transpose(pt[:h, :], src_ap, ident[:, :])
            if _tp_ctr[0]:
                nc.scalar.copy(out=dst_ap, in_=pt[:h, :])
            else:
                nc.vector.tensor_copy(out=dst_ap, in_=pt[:h, :])

        def gather(dst, tblap, idx_tile, t):
            nc.gpsimd.indirect_dma_start(
                out=dst, out_offset=None, in_=tblap,
                in_offset=bass.IndirectOffsetOnAxis(ap=idx_tile[:, t:t + 1], axis=0))

        def build_sel(slot_meta, first_meta, t):
            d = sb.tile([P, 1], FD, tag="seld", name="seld")
            nc.vector.tensor_tensor(out=d[:], in0=slot_meta[:, t:t + 1],
                                    in1=first_meta[:, t:t + 1], op=OP.subtract)
            sel = sb.tile([P, P], BFD, tag="sel", name="sel")
            nc.vector.tensor_tensor(out=sel[:], in0=d[:].to_broadcast([P, P]),
                                    in1=iota_sb[:], op=OP.is_equal)
            return sel

        def gmlp_chunk(rhs_list, w_in_tiles, w_out_tiles, scale_meta, base, tag):
            """Returns msg_im tiles (item-major [128,128] bf16) x4.
            scale_meta: None or meta tile [P, NT] whose cols base..base+3 give a
            per-item factor folded into msg."""
            hps = [PS([P, CH], f"h{m}") for m in range(4)]
            nk = len(w_in_tiles)
            for ki in range(nk):
                for m in range(4):
                    nc.tensor.matmul(hps[m][:], lhsT=w_in_tiles[ki][:, m * 128:(m + 1) * 128],
                                     rhs=rhs_list[ki], start=(ki == 0), stop=(ki == nk - 1))
            prods = []
            for j in range(2):
                sg = sb.tile([P, CH], BFD, tag=f"gm_sg{j}", name=f"{tag}sg{j}")
                nc.scalar.activation(sg[:], hps[2 + j][:], AF.Silu)
                pr = sbq.tile([P, CH], BFD, tag=f"gm_pr{j}", name=f"{tag}pr{j}")
                nc.vector.tensor_tensor(out=pr[:], in0=hps[j][:], in1=sg[:], op=OP.mult)
                prods.append(pr)
            ssps = PS([1, CH], "f")
            for j in range(2):
                sq = sb.tile([P, CH], BFD, tag=f"gm_sq{j}", name=f"{tag}sq{j}")
                nc.vector.tensor_tensor(out=sq[:], in0=prods[j][:], in1=prods[j][:],
                                        op=OP.mult)
                nc.tensor.matmul(ssps[:], lhsT=ones_sb[:], rhs=sq[:],
                                 start=(j == 0), stop=(j == 1))
            msr = sb.tile([1, CH], FD, tag="gm_ms", name=f"{tag}ms")
            nc.vector.tensor_scalar(out=msr[:], in0=ssps[:], scalar1=1.0 / 256,
                                    scalar2=1e-6, op0=OP.mult, op1=OP.add)
            rmsr = sb.tile([1, CH], FD, tag="gm_rm", name=f"{tag}rm")
            nc.scalar.activation(rmsr[:], msr[:], AF.Sqrt)
            invr = sb.tile([1, CH], FD, tag="gm_ir", name=f"{tag}ir")
            nc.vector.reciprocal(invr[:], rmsr[:])
            msgs = []
            for i in range(4):
                ip = PS([P, 1], "misc")
                nc.tensor.matmul(ip[:], lhsT=invr[:, i * 128:(i + 1) * 128],
                                 rhs=one1_sb[:], start=True, stop=True)
                iw = sb.tile([P, 1], FD, tag="gm_iw", name=f"{tag}iw")
                if scale_meta is not None:
                    nc.vector.tensor_tensor(out=iw[:], in0=ip[:],
                                            in1=scale_meta[:, base + i:base + i + 1],
                                            op=OP.mult)
                else:
                    nc.vector.tensor_copy(out=iw[:], in_=ip[:])
                mp = PS([P, P], "misc")
                for j in range(2):
                    nc.tensor.matmul(mp[:], lhsT=prods[j][:, i * 128:(i + 1) * 128],
                                     rhs=w_out_tiles[j][:], start=(j == 0), stop=(j == 1))
                mi = sbq.tile([P, P], BFD, tag="gm_mi", name=f"{tag}mi")
                nc.vector.tensor_scalar(out=mi[:], in0=mp[:], scalar1=iw[:],
                                        scalar2=None, op0=OP.mult)
                msgs.append(mi)
            return msgs

        def scatter_tile(sc_bf, scat_meta, t, tblA, tblB):
            tbl = tblA if (t % 2 == 0) else tblB
            nc.gpsimd.indirect_dma_start(
                out=tbl[:, :], out_offset=bass.IndirectOffsetOnAxis(
                    ap=scat_meta[:, t:t + 1], axis=0),
                in_=sc_bf, in_offset=None)

        # ================= P0: table1 + AG =================
        for b in range(NT_E):
            er = sb.tile([P, N_DIM], FD, tag="p0er", name="p0er")
            nc.sync.dma_start(out=er[:], in_=t_edge_res[b * P:(b + 1) * P, :])
            nm = rownorm(er[:], N_DIM, "p0")
            nc.sync.dma_start(out=t_tbl1_own[b * P:(b + 1) * P, :], in_=nm[:])
        nc.gpsimd.collective_compute(
            "AllGather", OP.bypass,
            ins=[t_tbl1_own[:, :].opt()], outs=[t_tbl1_full[:, :].opt()],
            replica_groups=[list(range(NCORES))])

        # ================= angle passes =================
        def angle_pass(pass3):
            w_in = wsb["w3_in"] if pass3 else wsb["w1_in"]
            w_out = [wsb["w3_out"][0][:], wsb["w3_out"][1][:]] if pass3 else \
                    [wsb["w1_out"][0][:], wsb["w1_out"][1][:]]
            tag = "a3" if pass3 else "a1"
            for ci in range(A_max // CH):
                t0 = ci * 4
                an_fm = sbf.tile([A_DIM, CH], BFD, tag="anf", name=tag + "anf")
                ar_tiles = []
                for i in range(4):
                    ar = sbq.tile([P, A_DIM], FD, tag="ar", name=tag + "ar")
                    nc.sync.dma_start(
                        out=ar[:], in_=t_angle_res[(t0 + i) * P:(t0 + i + 1) * P, :])
                    anm = rownorm(ar[:], A_DIM, tag + "an")
                    ar_tiles.append(ar)
                    transpose_to(an_fm[:, i * 128:(i + 1) * 128], anm[:], A_DIM)
                node_fm = sbf.tile([N_DIM, CH], BFD, tag="nf", name=tag + "nf")
                eij_fm = sbf.tile([N_DIM, CH], BFD, tag="jf", name=tag + "jf")
                eik_fm = sbf.tile([N_DIM, CH], BFD, tag="kf", name=tag + "kf")
                envs = []
                for i in range(4):
                    gn = sbg.tile([P, N_DIM], BFD, tag="gn", name=tag + "gn")
                    gather(gn[:], (t_nupd_full if pass3 else t_node_tbl)[:, :],
                           meta["a_n2a"], t0 + i)
                    transpose_to(node_fm[:, i * 128:(i + 1) * 128], gn[:], P)
                    if pass3:
                        gj = sbg.tile([P, 256], BFD, tag="gj2", name=tag + "gj")
                        gather(gj[:], t_tbl2_full[:, :], meta["a_eij"], t0 + i)
                        transpose_to(eij_fm[:, i * 128:(i + 1) * 128], gj[:, 0:128], P)
                        gk = sbg.tile([P, 256], BFD, tag="gk2", name=tag + "gk")
                        gather(gk[:], t_tbl2_full[:, :], meta["a_eik"], t0 + i)
                        transpose_to(eik_fm[:, i * 128:(i + 1) * 128], gk[:, 0:128], P)
                        env = sbq.tile([P, P], BFD, tag="env", name=tag + "env")
                        nc.vector.tensor_tensor(out=env[:], in0=gj[:, 128:256],
                                                in1=gk[:, 128:256], op=OP.mult)
                        envs.append(env)
                    else:
                        gj = sbg.tile([P, N_DIM], BFD, tag="gj1", name=tag + "gj")
                        gather(gj[:], t_tbl1_full[:, :], meta["a_eij"], t0 + i)
                        transpose_to(eij_fm[:, i * 128:(i + 1) * 128], gj[:], P)
                        gk = sbg.tile([P, N_DIM], BFD, tag="gk1", name=tag + "gk")
                        gather(gk[:], t_tbl1_full[:, :], meta["a_eik"], t0 + i)
                        transpose_to(eik_fm[:, i * 128:(i + 1) * 128], gk[:], P)
                rhs = [an_fm[:], node_fm[:], eij_fm[:], eik_fm[:]]
                e_fm = None
                if not pass3:
                    fps = PS([P, CH], "f")
                    gts = wsb["g1"]
                    for ki in range(4):
                        nc.tensor.matmul(fps[:], lhsT=gts[ki][:], rhs=rhs[ki],
                                         start=(ki == 0), stop=(ki == 3))
                    e_fm = sb.tile([P, CH], BFD, tag="efm", name=tag + "ef")
                    nc.scalar.activation(e_fm[:], fps[:], AF.Exp)
                msgs = gmlp_chunk(rhs, w_in, w_out,
                                  None if pass3 else meta["a_sw"], t0, tag)
                for i in range(4):
                    t = t0 + i
                    if not pass3:
                        ep = psp.tile([P, P], BFD, tag="tp2", name="ps_tp2", space="PSUM")
                        nc.tensor.transpose(ep[:], e_fm[:, i * 128:(i + 1) * 128],
                                            ident[:])
                        et = sb.tile([P, 256], BFD, tag="et", name=tag + "et")
                        nc.scalar.copy(out=et[:, 0:128], in_=ep[:])
                        nc.vector.tensor_tensor(out=et[:, 128:256], in0=et[:, 0:128],
                                                in1=msgs[i][:], op=OP.mult)
                        src = et
                        width = 256
                    else:
                        t3a = sb.tile([P, 128], BFD, tag="t3a", name=tag + "t3a")
                        nc.vector.tensor_tensor(out=t3a[:], in0=msgs[i][:],
                                                in1=envs[i][:], op=OP.mult)
                        t3 = sb.tile([P, 128], BFD, tag="t3", name=tag + "t3")
                        nc.vector.tensor_scalar(out=t3[:], in0=t3a[:],
                                                scalar1=meta["a_sw"][:, t:t + 1],
                                                scalar2=None, op0=OP.mult)
                        src = t3
                        width = 128
                    sel = build_sel(meta["a_slot"], meta["a_first"], t)
                    scp = PS([P, 256], "misc")
                    nc.tensor.matmul(scp[:, :width], lhsT=sel[:], rhs=src[:],
                                     start=True, stop=True)
                    scb = sb.tile([P, width], BFD, tag="scb" + str(width),
                                  name=tag + "scb")
                    nc.scalar.copy(out=scb[:], in_=scp[:, :width])
                    scatter_tile(scb[:], meta["a_scat"], t, t_accA, t_accB)
                    if pass3:
                        auf = sb.tile([P, P], BFD, tag="auf", name=tag + "auf")
                        transpose_to(auf[:], msgs[i][:], P)
                        aop = PS([P, A_DIM], "misc")
                        nc.tensor.matmul(aop[:], lhsT=auf[:], rhs=wsb["w3_aproj"][0][:],
                                         start=True, stop=True)
                        aout = sb.tile([P, A_DIM], FD, tag="aout", name=tag + "aout")
                        nc.vector.tensor_tensor(out=aout[:], in0=aop[:],
                                                in1=ar_tiles[i][:], op=OP.add)
                        nc.sync.dma_start(out=t_angle_out[t * P:(t + 1) * P, :],
                                          in_=aout[:])

        angle_pass(pass3=False)

        # ================= consume edge blocks =================
        def consume_edge_blocks(dst_t, src_res_t, pass3):
            for b in range(NT_E):
                ga = sb.tile([P, 256], BFD, tag="cba", name="cba")
                nc.sync.dma_start(out=ga[:], in_=t_accA[b * P:(b + 1) * P, :])
                gb = sb.tile([P, 256], BFD, tag="cbb", name="cbb")
                nc.sync.dma_start(out=gb[:], in_=t_accB[b * P:(b + 1) * P, :])
                accf = sb.tile([P, 256], FD, tag="cbf", name="cbf")
                nc.vector.tensor_tensor(out=accf[:], in0=ga[:], in1=gb[:], op=OP.add)
                res = sb.tile([P, N_DIM], FD, tag="cbr", name="cbr")
                nc.sync.dma_start(out=res[:], in_=src_res_t[b * P:(b + 1) * P, :])
                if not pass3:
                    s_eps = sb.tile([P, 128], FD, tag="cbs", name="cbs")
                    nc.vector.tensor_scalar(out=s_eps[:], in0=accf[:, 0:128],
                                            scalar1=1e-12, scalar2=None, op0=OP.add)
                    srec = sb.tile([P, 128], FD, tag="cbrc", name="cbrc")
                    nc.vector.reciprocal(srec[:], s_eps[:])
                    delta = sb.tile([P, 128], FD, tag="cbd", name="cbd")
                    nc.vector.tensor_tensor(out=delta[:], in0=accf[:, 128:256],
                                            in1=srec[:], op=OP.mult)
                else:
                    dbf = sb.tile([P, 128], BFD, tag="cdb", name="cdb")
                    nc.vector.tensor_copy(out=dbf[:], in_=accf[:, 0:128])
                    dfm = sb.tile([P, 128], BFD, tag="cdf", name="cdf")
                    transpose_to(dfm[:], dbf[:], P)
                    pp = PS([P, 128], "misc")
                    nc.tensor.matmul(pp[:], lhsT=dfm[:], rhs=wsb["w3_eproj"][0][:],
                                     start=True, stop=True)
                    delta = sb.tile([P, 128], FD, tag="cbd", name="cbd")
                    nc.vector.tensor_scalar(out=delta[:], in0=pp[:], scalar1=A_SCALE,
                                            scalar2=None, op0=OP.mult)
                out = sb.tile([P, N_DIM], FD, tag="cbo", name="cbo")
                nc.vector.tensor_tensor(out=out[:], in0=res[:], in1=delta[:],
                                        op=OP.add)
                nc.sync.dma_start(out=dst_t[b * P:(b + 1) * P, :], in_=out[:])

        consume_edge_blocks(t_eres1, t_edge_res, pass3=False)

        # ================= node tables =================
        def make_node_table(src_t, dst_t):
            for b in range(NBLK_N):
                nr = sb.tile([P, N_DIM], FD, tag="ntr", name="ntr")
                if b * P < npc:
                    nc.sync.dma_start(out=nr[:], in_=src_t[b * P:(b + 1) * P, :])
                else:
                    nc.vector.memset(nr[:], 0.0)
                nm = rownorm(nr[:], N_DIM, "nt")
                nc.sync.dma_start(out=dst_t[b * P:(b + 1) * P, :], in_=nm[:])

        make_node_table(t_node_res, t_nodeN)

        # ================= edge passes =================
        def edge_pass(pass4):
            tag = "e4" if pass4 else "e2"
            w_in = wsb["w4_in"] if pass4 else wsb["w2a_in"]
            w_out = [wsb["w4_out"][0][:], wsb["w4_out"][1][:]] if pass4 else \
                    [wsb["w2a_out"][0][:], wsb["w2a_out"][1][:]]
            src_res = t_eres3 if pass4 else t_eres1
            node_src = t_nodeN4 if pass4 else t_nodeN
            for ci in range(E_max // CH):
                t0 = ci * 4
                rbf_ch = sbg.tile([R_DIM, CH], BFD, tag="rbfc", name=tag + "rbf")
                nc.sync.dma_start(out=rbf_ch[:], in_=t_rbf_t[:, ci * CH:(ci + 1) * CH])
                en_fm = sbf.tile([N_DIM, CH], BFD, tag="ef", name=tag + "ef")
                res_tiles = []
                for i in range(4):
                    er = sbq.tile([P, N_DIM], FD, tag="er", name=tag + "er")
                    nc.sync.dma_start(
                        out=er[:], in_=src_res[(t0 + i) * P:(t0 + i + 1) * P, :])
                    enm = rownorm(er[:], N_DIM, tag + "en")
                    res_tiles.append(er)
                    transpose_to(en_fm[:, i * 128:(i + 1) * 128], enm[:], P)
                node_fm = sbf.tile([N_DIM, CH], BFD, tag="nf", name=tag + "nf")
                ext_fm = sbf.tile([N_DIM, CH], BFD, tag="xf", name=tag + "xf")
                for i in range(4):
                    gn = sbg.tile([P, N_DIM], BFD, tag="gn", name=tag + "gn")
                    gather(gn[:], node_src[:, :], meta["e_gnode"], t0 + i)
                    transpose_to(node_fm[:, i * 128:(i + 1) * 128], gn[:], P)
                    gx = sbg.tile([P, N_DIM], BFD, tag="gx", name=tag + "gx")
                    gather(gx[:], t_ext_tbl[:, :], meta["e_gext"], t0 + i)
                    transpose_to(ext_fm[:, i * 128:(i + 1) * 128], gx[:], P)
                rhs = [en_fm[:], node_fm[:], ext_fm[:]]
                e_fm = None
                if not pass4:
                    fps = PS([P, CH], "f")
                    nc.tensor.matmul(fps[:], lhsT=wsb["g2"][0][:], rhs=en_fm[:],
                                     start=True, stop=True)
                    e_fm = sb.tile([P, CH], BFD, tag="efm", name=tag + "efm")
                    nc.scalar.activation(e_fm[:], fps[:], AF.Exp)
                msgs = gmlp_chunk(rhs, w_in, w_out,
                                  None if pass4 else meta["e_sw"], t0, tag)
                for i in range(4):
                    t = t0 + i
                    if not pass4:
                        ep = psp.tile([P, P], BFD, tag="tp2", name="ps_tp2", space="PSUM")
                        nc.tensor.transpose(ep[:], e_fm[:, i * 128:(i + 1) * 128],
                                            ident[:])
                        et = sb.tile([P, 256], BFD, tag="et", name=tag + "et")
                        nc.scalar.copy(out=et[:, 0:128], in_=ep[:])
                        nc.vector.tensor_tensor(out=et[:, 128:256], in0=et[:, 0:128],
                                                in1=msgs[i][:], op=OP.mult)
                        src = et
                        width = 256
                    else:
                        evp = PS([P, P], "misc")
                        nc.tensor.matmul(evp[:], lhsT=rbf_ch[:, i * P:(i + 1) * P],
                                         rhs=wsb["w4_env"][0][:], start=True, stop=True)
                        ev = sb.tile([P, P], BFD, tag="evs", name=tag + "evs")
                        nc.scalar.activation(ev[:], evp[:], AF.Sigmoid)
                        t4a = sb.tile([P, 128], BFD, tag="t4a", name=tag + "t4a")
                        nc.vector.tensor_tensor(out=t4a[:], in0=msgs[i][:],
                                                in1=ev[:], op=OP.mult)
                        t4 = sb.tile([P, 128], BFD, tag="t4", name=tag + "t4")
                        nc.vector.tensor_scalar(out=t4[:], in0=t4a[:],
                                                scalar1=meta["e_sw"][:, t:t + 1],
                                                scalar2=None, op0=OP.mult)
                        src = t4
                        width = 128
                    sel = build_sel(meta["e_slot"], meta["e_first"], t)
                    scp = PS([P, 256], "misc")
                    nc.tensor.matmul(scp[:, :width], lhsT=sel[:], rhs=src[:],
                                     start=True, stop=True)
                    scb = sb.tile([P, width], BFD, tag="scb" + str(width),
                                  name=tag + "scb")
                    nc.scalar.copy(out=scb[:], in_=scp[:, :width])
                    scatter_tile(scb[:], meta["e_scat"], t, t_naccA, t_naccB)
                    if pass4:
                        nf2 = sb.tile([P, P], BFD, tag="nf2", name=tag + "nf2")
                        transpose_to(nf2[:], msgs[i][:], P)
                        pp = PS([P, 128], "misc")
                        nc.tensor.matmul(pp[:], lhsT=nf2[:], rhs=wsb["w4_eproj"][0][:],
                                         start=True, stop=True)
                        eo = sb.tile([P, N_DIM], FD, tag="eo", name=tag + "eo")
                        nc.vector.tensor_tensor(out=eo[:], in0=res_tiles[i][:],
                                                in1=pp[:], op=OP.add)
                        nc.sync.dma_start(out=t_edge_out[t * P:(t + 1) * P, :],
                                          in_=eo[:])
                if not pass4:
                    msgs_e = gmlp_chunk(rhs, wsb["w2e_in"],
                                        [wsb["w2e_out"][0][:], wsb["w2e_out"][1][:]],
                                        meta["e_sw"], t0, tag + "b")
                    for i in range(4):
                        t = t0 + i
                        r2 = sb.tile([P, N_DIM], FD, tag="r2", name=tag + "r2")
                        nc.vector.tensor_tensor(out=r2[:], in0=res_tiles[i][:],
                                                in1=msgs_e[i][:], op=OP.add)
                        nc.sync.dma_start(out=t_eres2[t * P:(t + 1) * P, :], in_=r2[:])
                        nm2 = rownorm(r2[:], N_DIM, tag + "n2")
                        nc.sync.dma_start(out=t_tbl2_own[t * P:(t + 1) * P, 0:128],
                                          in_=nm2[:])
                        evp = PS([P, P], "misc")
                        nc.tensor.matmul(evp[:], lhsT=rbf_ch[:, i * P:(i + 1) * P],
                                         rhs=wsb["w3_env"][0][:], start=True, stop=True)
                        ev = sb.tile([P, P], BFD, tag="ev3", name=tag + "ev3")
                        nc.scalar.activation(ev[:], evp[:], AF.Sigmoid)
                        nc.sync.dma_start(out=t_tbl2_own[t * P:(t + 1) * P, 128:256],
                                          in_=ev[:])

        edge_pass(pass4=False)

        # ================= consume node blocks =================
        def consume_node_blocks(dst_t, src_t, pass4):
            for b in range(npc // P):
                ga = sb.tile([P, 256], BFD, tag="nba", name="nba")
                nc.sync.dma_start(out=ga[:], in_=t_naccA[b * P:(b + 1) * P, :])
                gb = sb.tile([P, 256], BFD, tag="nbb", name="nbb")
                nc.sync.dma_start(out=gb[:], in_=t_naccB[b * P:(b + 1) * P, :])
                accf = sb.tile([P, 256], FD, tag="nbf", name="nbf")
                nc.vector.tensor_tensor(out=accf[:], in0=ga[:], in1=gb[:], op=OP.add)
                res = sb.tile([P, N_DIM], FD, tag="nbr", name="nbr")
                nc.sync.dma_start(out=res[:], in_=src_t[b * P:(b + 1) * P, :])
                if not pass4:
                    s_eps = sb.tile([P, 128], FD, tag="nbs", name="nbs")
                    nc.vector.tensor_scalar(out=s_eps[:], in0=accf[:, 0:128],
                                            scalar1=1e-12, scalar2=None, op0=OP.add)
                    srec = sb.tile([P, 128], FD, tag="nbrc", name="nbrc")
                    nc.vector.reciprocal(srec[:], s_eps[:])
                    delta = sb.tile([P, 128], FD, tag="nbd", name="nbd")
                    nc.vector.tensor_tensor(out=delta[:], in0=accf[:, 128:256],
                                            in1=srec[:], op=OP.mult)
                else:
                    dbf = sb.tile([P, 128], BFD, tag="ndb", name="ndb")
                    nc.vector.tensor_copy(out=dbf[:], in_=accf[:, 0:128])
                    dfm = sb.tile([P, 128], BFD, tag="ndf", name="ndf")
                    transpose_to(dfm[:], dbf[:], P)
                    pp = PS([P, 128], "misc")
                    nc.tensor.matmul(pp[:], lhsT=dfm[:], rhs=wsb["w4_nproj"][0][:],
                                     start=True, stop=True)
                    delta = sb.tile([P, 128], FD, tag="nbd", name="nbd")
                    nc.vector.tensor_scalar(out=delta[:], in0=pp[:], scalar1=E_SCALE,
                                            scalar2=None, op0=OP.mult)
                out = sb.tile([P, N_DIM], FD, tag="nbo", name="nbo")
                nc.vector.tensor_tensor(out=out[:], in0=res[:], in1=delta[:], op=OP.add)
                nc.sync.dma_start(out=dst_t[b * P:(b + 1) * P, :], in_=out[:])

        consume_node_blocks(t_nres1, t_node_res, pass4=False)

        for b in range(npc // P):
            nr = sb.tile([P, N_DIM], FD, tag="nu", name="nu")
            nc.sync.dma_start(out=nr[:], in_=t_nres1[b * P:(b + 1) * P, :])
            nb = sb.tile([P, N_DIM], BFD, tag="nub", name="nub")
            nc.vector.tensor_copy(out=nb[:], in_=nr[:])
            nc.sync.dma_start(out=t_nupd_own[b * P:(b + 1) * P, :], in_=nb[:])

        nc.gpsimd.collective_compute(
            "AllGather", OP.bypass,
            ins=[t_tbl2_own[:, :].opt()], outs=[t_tbl2_full[:, :].opt()],
            replica_groups=[list(range(NCORES))])
        nc.gpsimd.collective_compute(
            "AllGather", OP.bypass,
            ins=[t_nupd_own[:, :].opt()], outs=[t_nupd_full[:, :].opt()],
            replica_groups=[list(range(NCORES))])

        angle_pass(pass3=True)
        consume_edge_blocks(t_eres3, t_eres2, pass3=True)
        make_node_table(t_nres1, t_nodeN4)
        edge_pass(pass4=True)
        consume_node_blocks(t_node_out, t_nres1, pass4=True)

    nc.compile()
    return nc


# ----------------------------------------------------------------------------
# entry point
# ----------------------------------------------------------------------------

def kernel(_return_parts=False, **inputs):
    nframes = int(inputs["nframes"])
    nloc = int(inputs["nloc"])
    num_nodes = nframes * nloc
    node_ebd_ext = np.asarray(inputs["node_ebd_ext"], F32)
    edge_ebd = np.asarray(inputs["edge_ebd"], F32)
    angle_ebd = np.asarray(inputs["angle_ebd"], F32)
    sw = np.asarray(inputs["sw"], F32)
    a_sw = np.asarray(inputs["a_sw"], F32)
    edge_index = np.asarray(inputs["edge_index"], np.int64)
    angle_index = np.asarray(inputs["angle_index"], np.int64)
    edge_rbf = np.asarray(inputs["edge_rbf"], F32)

    n2e, n_ext2e = edge_index[0], edge_index[1]
    n2a, eij2a, eik2a = angle_index[0], angle_index[1], angle_index[2]
    node_ebd = node_ebd_ext[:, :nloc, :].reshape(-1, N_DIM)
    node_ext = node_ebd_ext.reshape(-1, N_DIM)

    plan = build_plan(n2e, n_ext2e, n2a, eij2a, eik2a, num_nodes)
    npc, E_max, A_max = plan["npc"], plan["E_max"], plan["A_max"]
    NT_A, NT_E = A_max // P, E_max // P
    NODE_TBL = npc + P
    ETBL = E_max + P

    W = fold_weights(inputs["params"])

    nc = build_kernel(E_max, A_max, npc)

    IOTA = np.broadcast_to(np.arange(P, dtype=F32)[None, :], (P, P)).copy()
    ONES = np.ones((P, 1), BF)
    ONE1 = np.ones((1, 1), F32)
    node_tbl = node_ebd.astype(BF)
    ext_tbl = node_ext.astype(BF)

    in_maps = []
    for c in range(NCORES):
        eids = plan["edge_lists"][c]
        ke = len(eids)
        s2, cols = plan["per_core_a"][c]
        av = s2 >= 0

        edge_res = np.zeros((E_max, N_DIM), F32)
        edge_res[:ke] = edge_ebd[eids]
        angle_res = np.zeros((A_max, A_DIM), F32)
        angle_res[av] = angle_ebd[cols["aid"][av]]
        node_res = node_ebd[c * npc:(c + 1) * npc].astype(F32)
        a_sw_arr = np.zeros(A_max, F32)
        a_sw_arr[av] = a_sw[cols["aid"][av]]
        sw_arr = np.zeros(E_max, F32)
        sw_arr[:ke] = sw[eids]
        rbf_t = np.zeros((R_DIM, E_max), BF)
        rbf_t[:, :ke] = edge_rbf[eids].T.astype(BF)

        n2a_idx = np.where(av, cols["n2a"], 0).astype(np.int64)
        eik_g = np.where(av, cols["eik_core"] * E_max + cols["eik_slot"], 0)
        eij_g = np.where(av, c * E_max + cols["eij_slot_g"], 0)
        a_slot = s2.astype(F32)
        a_first, a_scat = _scatter_plan(s2, NT_A, E_max)  # trash row = E_max..E_max+127
        a_first_b = np.broadcast_to(a_first[None, :], (P, NT_A)).astype(F32)

        n2e_loc = np.full(E_max, -1.0, F32)
        n2e_loc[:ke] = (n2e[eids] - c * npc).astype(F32)
        e_slot_i = np.full(E_max, -1, np.int64)
        e_slot_i[:ke] = n2e[eids] - c * npc
        gnode_idx = np.where(e_slot_i >= 0, e_slot_i, 0)
        gext_idx = np.zeros(E_max, np.int64)
        gext_idx[:ke] = n_ext2e[eids]
        e_first, e_scat = _scatter_plan(e_slot_i, NT_E, NODE_TBL)
        e_first_b = np.broadcast_to(e_first[None, :], (P, NT_E)).astype(F32)

        im = {
            "angle_res": angle_res, "edge_res": edge_res, "node_res": node_res,
            "node_tbl": node_tbl, "ext_tbl": ext_tbl, "rbf_t": rbf_t,
            "a_n2a": _blocked_i32(n2a_idx, NT_A),
            "a_eik": _blocked_i32(eik_g, NT_A),
            "a_eij": _blocked_i32(eij_g, NT_A),
            "a_slot": _blocked_f32(a_slot, NT_A),
            "a_first": np.ascontiguousarray(a_first_b),
            "a_scat": _blocked_i32(a_scat, NT_A),
            "a_sw_b": _blocked_f32(a_sw_arr, NT_A),
            "e_gnode": _blocked_i32(gnode_idx, NT_E),
            "e_gext": _blocked_i32(gext_idx, NT_E),
            "e_slot": _blocked_f32(n2e_loc, NT_E),
            "e_first": np.ascontiguousarray(e_first_b),
            "e_scat": _blocked_i32(e_scat, NT_E),
            "e_sw_b": _blocked_f32(sw_arr, NT_E),
            "iota": IOTA, "ones_col": ONES, "one_one": ONE1,
            "accA": np.zeros((ETBL, 256), BF),
            "accB": np.zeros((ETBL, 256), BF),
            "naccA": np.zeros((NODE_TBL + P, 256), BF),
            "naccB": np.zeros((NODE_TBL + P, 256), BF),
        }
        for k, v in W.items():
            im["W_" + k] = v
        in_maps.append(im)

    def stitch(results):
        node_out = np.zeros((num_nodes, N_DIM), F32)
        edge_out = np.zeros_like(edge_ebd)
        angle_out = np.zeros_like(angle_ebd)
        for c in range(NCORES):
            r = results[c]
            eids = plan["edge_lists"][c]
            s2, cols = plan["per_core_a"][c]
            av = s2 >= 0
            node_out[c * npc:(c + 1) * npc] = r["node_out"]
            edge_out[eids] = r["edge_out"][:len(eids)]
            angle_out[cols["aid"][av]] = r["angle_out"][av]
        return (node_out.reshape(nframes, nloc, N_DIM), edge_out, angle_out)

    if _return_parts:
        return nc, in_maps, stitch
    res = bass_utils.run_bass_kernel_spmd(nc, in_maps, core_ids=list(range(NCORES)))
    return stitch(res.results)
